# revision 14
# baseline (speedup 1.0000x reference)
"""Trainium2 Bass kernel: per-timestep dense softmax attention (frame + memory).

Problem (hardcoded): B=2, T=8, HW=4096, C=64, Cv=3, M=1024, fp32.
  out[b,t] = 0.8 * softmax(kj @ ki^T) @ vi  +  0.2 * softmax(kj @ mk^T) @ mv
with kj = k[b,t+1] (queries), ki = k[b,t] (keys), vi = v[b,t].

Sharding: 8 cores = 2 batches x 4 query-blocks of 1024 rows. Each core handles
all 7 timesteps for its (b, q-range).

v5 design:
  - Keys/queries/memory-keys transposed on the HOST to [C, n] bf16 layout:
    zero device transposes, zero casts, half the DMA bytes.
  - One chunk (128 keys) per inner iteration: logits^T tile [128, 1024 q]
    via 2x 512-col bf16 matmuls on PE rows 0:64. No row packing: the
    single-tile-per-iteration PSUM rotation (3 slots) gives the exp engines
    ~3 iterations of drain slack, which row packing would halve.
  - exp split across two engines: ~21 chunks/step on ACT (native Exp),
    ~19 on DVE via Schraudolph fast-exp (one tensor_scalar:
    i16 = A*x + B, bitcast to bf16).
  - AV lhsT is 8 wide: frame chunks carry (v, 1, 0,0,0,0) -> rows 0:4,
    memory chunks (0,0,0,0, mv, 1) -> rows 4:8 of the same accumulator
    group, so frame+memory share col-groups and the 4 PSUM col-groups key
    on (chunk parity, q-half): 4-way concurrent AV, batched 2 chunks at a
    time one iteration behind the logits.
  - normalization + 0.8/0.2 combine + transpose to [q, 3] done on host.
"""

import numpy as np

import concourse.bacc as bacc
import concourse.bass as bass
import concourse.tile as tile
from concourse import masks, mybir
from concourse.bass_utils import run_bass_kernel_spmd

B, T, HW, C, Cv, M = 2, 8, 4096, 64, 3, 1024
TS = T - 1  # 7 steps
QB = HW // 4  # 1024 queries per core
NKC = HW // 128  # 32 frame key chunks
NMC = M // 128  # 8 memory key chunks
NPAIR = (NKC + NMC) // 2  # 20 chunk pairs per step
COEF = 0.2

# Schraudolph fast-exp constants: bf16bits(e^x) ~= int16(EXP_A*x + EXP_B)
EXP_A = 184.6649652337873
EXP_B = 16250.4

F32 = mybir.dt.float32
BF16 = mybir.dt.bfloat16
I16 = mybir.dt.int16
AF = mybir.ActivationFunctionType
ALU = mybir.AluOpType

_CACHE = {}


def _build_nc(repeat=1, mode="full"):
    nc = bacc.Bacc("TRN2", target_bir_lowering=False)

    kT = nc.dram_tensor("kT", [TS, C, HW], BF16, kind="ExternalInput")
    qT = nc.dram_tensor("qT", [TS, C, QB], BF16, kind="ExternalInput")
    mkT = nc.dram_tensor("mkT", [TS, C, M], BF16, kind="ExternalInput")
    vf = nc.dram_tensor("vf", [TS, HW, Cv], BF16, kind="ExternalInput")
    mv = nc.dram_tensor("mv", [TS, M, Cv], BF16, kind="ExternalInput")
    out = nc.dram_tensor("out", [TS, 16, QB], F32, kind="ExternalOutput")

    def mm(o, lhsT, rhs, **kw):
        nc.tensor.matmul(o, lhsT=lhsT, rhs=rhs, **kw)

    with tile.TileContext(nc) as tc:
        with (
            tc.tile_pool(name="singles", bufs=1) as singles,
            tc.tile_pool(name="ktp", bufs=3) as kt_p,
            tc.tile_pool(name="qtp", bufs=3) as qt_p,
            tc.tile_pool(name="mktp", bufs=3) as mkt_p,
            tc.tile_pool(name="expp", bufs=16) as exp_p,
            tc.tile_pool(name="ostage", bufs=2) as ost_p,
            tc.tile_pool(name="ps_l", bufs=3, space="PSUM") as ps_l_p,
            tc.tile_pool(name="ps_acc", bufs=1, space="PSUM") as ps_acc_p,
        ):
            # 8-wide AV weights: frame chunks (v0,v1,v2,1,0,0,0,0),
            # memory chunks (0,0,0,0,mv0,mv1,mv2,1) -> frame sums land in
            # accumulator rows 0:4 and memory sums in rows 4:8 of the same
            # col-group.
            v1 = singles.tile([128, TS, NKC, 8], BF16)
            mv1 = singles.tile([128, TS, NMC, 8], BF16)
            nc.vector.memset(v1[:], 0.0)
            nc.vector.memset(mv1[:], 0.0)
            nc.vector.memset(v1[:, :, :, 3:4], 1.0)
            nc.vector.memset(mv1[:, :, :, 7:8], 1.0)
            for t in range(TS):
                nc.gpsimd.dma_start(
                    out=v1[:, t, :, 0:3],
                    in_=vf[t].rearrange("(c p) d -> p c d", p=128),
                )
                nc.gpsimd.dma_start(
                    out=mv1[:, t, :, 4:7],
                    in_=mv[t].rearrange("(c p) d -> p c d", p=128),
                )

            if mode == "dma":
                ost0 = ost_p.tile([128, QB], F32, tag="ost")
                nc.vector.memset(ost0[:], 0.0)

            for _rep in range(repeat):
              for t in range(TS):
                # ---- load this step's keys/queries/memories, duplicated
                # into both partition halves so each chunk's logits can be
                # computed as two concurrent half-array matmuls (full-array
                # activity also keeps the HAM clock gate at 8/8) ----
                KT = kt_p.tile([128, HW], BF16, tag="kt")
                QT = qt_p.tile([128, QB], BF16, tag="qt")
                MKT = mkt_p.tile([128, M], BF16, tag="mkt")
                nc.sync.dma_start(out=KT[0:64, :], in_=kT[t])
                nc.sync.dma_start(out=KT[64:128, :], in_=kT[t])
                nc.sync.dma_start(out=QT[0:64, :], in_=qT[t])
                nc.sync.dma_start(out=QT[64:128, :], in_=qT[t])
                nc.sync.dma_start(out=MKT[0:64, :], in_=mkT[t])
                nc.sync.dma_start(out=MKT[64:128, :], in_=mkT[t])
                if mode == "dma":
                    for i in range(2):
                        nc.sync.dma_start(
                            out=out[t, 8 * i : 8 * i + 8],
                            in_=ost0[8 * i : 8 * i + 8, :],
                        )
                    continue

                acc = ps_acc_p.tile([128, QB], F32, tag="acc")
                NCH = NKC + NMC  # 40 chunks: 0..31 frame, 32..39 memory

                def emit_av(cs):
                    # batched AV for two chunks, col-tiled on chunk parity
                    # (2-way concurrent); emitted one iteration behind the
                    # logits so the exp sems are already satisfied.
                    for h in range(2):
                        sl = slice(h * 512, (h + 1) * 512)
                        for c, ex in cs:
                            fr = c < NKC
                            lv = (v1[:, t, c, :] if fr
                                  else mv1[:, t, c - NKC, :])
                            g = (c % 2) + 2 * (t % 2)
                            start = c < 2
                            stop = c >= NCH - 2
                            mm(
                                acc[32 * g : 32 * g + 8, sl],
                                lv,
                                ex[:, sl],
                                start=start,
                                stop=stop,
                                tile_position=(0, 32 * g),
                                skip_group_check=True,
                            )

                pend = []
                for c in range(NCH):
                    fr = c < NKC
                    src = KT if fr else MKT
                    cc = c if fr else c - NKC
                    psl = ps_l_p.tile([128, QB], F32, tag="psl")
                    # q-halves on the two PE row halves, concurrently
                    mm(
                        psl[:, 0:512],
                        src[0:64, cc * 128 : cc * 128 + 128],
                        QT[0:64, 0:512],
                        start=True, stop=True,
                    )
                    mm(
                        psl[:, 512:1024],
                        src[64:128, cc * 128 : cc * 128 + 128],
                        QT[64:128, 512:1024],
                        start=True, stop=True,
                        tile_position=(64, 0),
                    )
                    if mode == "logits":
                        continue
                    if len(pend) == 4 and mode == "full":
                        emit_av(pend[:2])
                        pend = pend[2:]

                    # ---- exp: ~21 chunks on ACT, ~19 on DVE (fast exp) ----
                    ex = exp_p.tile([128, QB], BF16, tag="ex")
                    if c % 2 == 0 or c == 1:
                        nc.scalar.activation(ex[:], psl[:], AF.Exp)
                    else:
                        nc.vector.tensor_scalar(
                            ex[:].bitcast(I16), psl[:], EXP_A, EXP_B,
                            ALU.mult, ALU.add,
                        )
                    pend.append((c, ex))
                if mode in ("logits", "exp"):
                    continue
                emit_av(pend[:2])
                emit_av(pend[2:])

                ost = ost_p.tile([128, QB], F32, tag="ost")
                nc.vector.tensor_copy(ost[:], acc[:])
                base = 64 * (t % 2)
                for i in range(2):
                    nc.sync.dma_start(
                        out=out[t, 8 * i : 8 * i + 8],
                        in_=ost[base + 32 * i : base + 32 * i + 8, :],
                    )
    nc.finalize()
    return nc


def make_in_maps(k, v, m_k, m_v):
    import ml_dtypes

    bf = ml_dtypes.bfloat16
    k = np.asarray(k, dtype=np.float32)
    v = np.asarray(v, dtype=np.float32)
    m_k = np.asarray(m_k, dtype=np.float32)
    m_v = np.asarray(m_v, dtype=np.float32)
    in_maps = []
    kTb = [np.ascontiguousarray(k[b, :-1].transpose(0, 2, 1)).astype(bf)
           for b in range(B)]
    mkTb = [np.ascontiguousarray(m_k[b].transpose(0, 2, 1)).astype(bf)
            for b in range(B)]
    vfb = [np.ascontiguousarray(v[b, :-1]).astype(bf) for b in range(B)]
    mvb = [np.ascontiguousarray(m_v[b]).astype(bf) for b in range(B)]
    for core in range(8):
        b, qc = core // 4, core % 4
        qsl = slice(qc * QB, (qc + 1) * QB)
        in_maps.append({
            "kT": kTb[b],
            "qT": np.ascontiguousarray(
                k[b, 1:, qsl, :].transpose(0, 2, 1)).astype(bf),
            "mkT": mkTb[b],
            "vf": vfb[b],
            "mv": mvb[b],
        })
    return in_maps


def _combine(outp, res_core, core):
    b, qc = core // 4, core % 4
    o = res_core  # [TS, 16, QB]: 2 groups x 8 rows (chunk parity)
    tot = o[:, 0:8] + o[:, 8:16]  # rows 0:3 fnum, 3 fden, 4:7 mnum, 7 mden
    nk, dk = tot[:, 0:3], tot[:, 3]
    nm, dm = tot[:, 4:7], tot[:, 7]
    rec = (1.0 - COEF) * nk / dk[:, None, :] + COEF * nm / dm[:, None, :]
    outp[b, :, qc * QB : (qc + 1) * QB, :] = rec.transpose(0, 2, 1)


def _make_sharded(nc, n_cores=8):
    """Build the shard_map'd jitted callable once, mirroring
    bass2jax.run_bass_via_pjrt, so repeated timed executions reuse the
    compiled executable and device-resident inputs."""
    import jax
    import jax.numpy as jnp
    from jax.sharding import Mesh, PartitionSpec
    from jax.experimental.shard_map import shard_map
    from concourse import bass2jax, mybir as _mybir

    bass2jax.install_neuronx_cc_hook()
    partition_name = (
        nc.partition_id_tensor.name if nc.partition_id_tensor else None
    )
    in_names, out_names, out_avals, zero_outs = [], [], [], []
    for alloc in nc.m.functions[0].allocations:
        if not isinstance(alloc, mybir.MemoryLocationSet):
            continue
        name = alloc.memorylocations[0].name
        if alloc.kind == "ExternalInput":
            if name != partition_name:
                in_names.append(name)
        elif alloc.kind == "ExternalOutput":
            out_names.append(name)
            shape = tuple(alloc.tensor_shape)
            dtype = _mybir.dt.np(alloc.dtype)
            out_avals.append(jax.core.ShapedArray(shape, dtype))
            zero_outs.append(np.zeros(shape, dtype))
    n_params = len(in_names)
    all_in_names = in_names + out_names
    if partition_name is not None:
        all_in_names.append(partition_name)
    donate = tuple(range(n_params, n_params + len(out_avals)))

    def _body(*args):
        operands = list(args)
        if partition_name is not None:
            operands.append(bass2jax.partition_id_tensor())
        outs = bass2jax._bass_exec_p.bind(
            *operands,
            out_avals=tuple(out_avals),
            in_names=tuple(all_in_names),
            out_names=tuple(out_names),
            lowering_input_output_aliases=(),
            sim_require_finite=True,
            sim_require_nnan=True,
            nc=nc,
        )
        return tuple(outs)

    devices = jax.devices()[:n_cores]
    mesh = Mesh(np.asarray(devices), ("core",))
    sharded = jax.jit(
        shard_map(
            _body, mesh=mesh,
            in_specs=(PartitionSpec("core"),) * (n_params + len(out_avals)),
            out_specs=(PartitionSpec("core"),) * len(out_names),
            check_rep=False,
        ),
        donate_argnums=donate,
        keep_unused=True,
    )
    return sharded, in_names, out_names, zero_outs


def bench(k, v, m_k, m_v, iters=30, repeat=1, mode="full"):
    """Time repeated on-device executions; returns per-iter seconds list."""
    import time as _time
    import jax

    key = f"nc{repeat}_{mode}"
    if key not in _CACHE:
        _CACHE[key] = _build_nc(repeat=repeat, mode=mode)
    nc = _CACHE[key]
    in_maps = make_in_maps(k, v, m_k, m_v)
    sharded, in_names, out_names, zero_outs = _make_sharded(nc)
    concat_in = [
        np.concatenate([np.asarray(in_maps[c][n]) for c in range(8)], axis=0)
        for n in in_names
    ]
    dev_in = [jax.device_put(a) for a in concat_in]  # resident once
    times = []
    out = None
    for i in range(iters + 3):
        zeros = [np.zeros((8 * z.shape[0], *z.shape[1:]), z.dtype) for z in zero_outs]
        dz = jax.block_until_ready([jax.device_put(z) for z in zeros])
        t0 = _time.perf_counter()
        out = jax.block_until_ready(sharded(*dev_in, *dz))
        t1 = _time.perf_counter()
        if i >= 3:
            times.append(t1 - t0)
    return times, out


def kernel(k, v, m_k, m_v):
    if "nc" not in _CACHE:
        _CACHE["nc"] = _build_nc()
    nc = _CACHE["nc"]
    in_maps = make_in_maps(k, v, m_k, m_v)
    res = run_bass_kernel_spmd(nc, in_maps, core_ids=list(range(8)))
    _CACHE["last_result"] = res

    outp = np.empty((B, TS, HW, Cv), dtype=np.float32)
    for core in range(8):
        _combine(outp, res.results[core]["out"], core)
    return outp
